# revision 1
# baseline (speedup 1.0000x reference)
"""Gaussian square-sensor splat on 8 Trainium2 NeuronCores (v3.1).

Decomposition: the 2048x2048 image is split into 64x64=4096 blocks of
32x32 px.  Each block is assigned to one of 8 cores by COUNT-BALANCED
DEALING: blocks sorted by point count, rank r -> core r%8, slot r//8.
The 8 blocks sharing a slot have near-identical counts, so one shared
program (slot capacities = ceil(max count in slot / 128)*128) serves all
cores SPMD with ~17% fewer point-tiles than fixed-capacity bucketing.

Per 128-point tile, the 5x5 Gaussian footprint is a rank-1 outer product
of row/column profiles over the block's 36x36 patch (halo 2), computed
as Derivative_Erf(sqrt(2) d) = (2/sqrt(pi)) exp(-2 d^2) in one fused
[P,G,2,36] activation pass, accumulated with PE matmuls into PSUM
strips (8 slots per strip), staged to SBUF, DMA'd out, and overlap-added
on the host.  Host precomputes patch offsets dcy/dcx and normalized
values vn = v / (2 (1+2q cos 2pi fy)(1+2q cos 2pi fx)) (Jacobi theta
row-sum normalization; the profiles' 4/pi constant is folded in).
"""
import math
import sys

sys.path.insert(0, '/opt/trn_rl_repo')

import numpy as np

WIDTH = HEIGHT = 2048
N_POINTS = 1 << 20
N_CORES = 8
BLK = 32
PW = 36
GRID = WIDTH // BLK                     # 64 blocks per side
NBLK = GRID * GRID                      # 4096
NSLOT = NBLK // N_CORES                 # 512 slots per core
NSTRIP = NSLOT // 8                     # 64 psum strips per core
P = 128

_Q2 = 2.0 * math.exp(-math.pi ** 2 / 2.0)

_COMPILED = None          # (nc, plan)


def _block_plan(x, y):
    """Assign blocks to (core, slot) by count-balanced dealing."""
    xp = (x.astype(np.float64) + 1.0) * (WIDTH / 2.0)
    yp = (y.astype(np.float64) + 1.0) * (HEIGHT / 2.0)
    xb = np.clip(np.floor(xp).astype(np.int64), 0, WIDTH - 1)
    yb = np.clip(np.floor(yp).astype(np.int64), 0, HEIGHT - 1)
    gb = (yb // BLK) * GRID + xb // BLK            # global block id
    counts = np.bincount(gb, minlength=NBLK)
    order = np.argsort(-counts, kind="stable")     # blocks by count desc
    core_of = np.empty(NBLK, np.int64)
    slot_of = np.empty(NBLK, np.int64)
    rank = np.arange(NBLK)
    core_of[order] = rank % N_CORES
    slot_of[order] = rank // N_CORES
    # slot capacity = max count within the slot's 8 blocks, 64-quantized
    slot_max = counts[order].reshape(NSLOT, N_CORES).max(axis=1)
    caps = (np.ceil(slot_max / 128).astype(np.int64) * 128).clip(128, None)
    # inverse table: (core, slot) -> block id
    inv = np.empty((N_CORES, NSLOT), np.int64)
    inv[core_of[order], slot_of[order]] = order
    return dict(counts=counts, core_of=core_of, slot_of=slot_of,
                caps=caps, inv=inv)


def _layout_from_caps(caps):
    """Slot slab layout: slots packed per strip (8 slots/strip), strips
    padded to whole 128-slot columns (caps are multiples of 128 so no
    padding actually occurs)."""
    strip_cols = np.zeros(NSTRIP, np.int64)
    slot_off = np.zeros(NSLOT, np.int64)     # slot offset within strip
    col_base = np.zeros(NSTRIP, np.int64)
    segs = []
    for s in range(NSTRIP):
        off = 0
        for j in range(8):
            sl = s * 8 + j
            slot_off[sl] = off
            off += caps[sl]
        strip_cols[s] = (off + 127) // 128
    col_base[1:] = np.cumsum(strip_cols)[:-1]
    F = int(strip_cols.sum())

    for s in range(NSTRIP):
        slist = []
        for j in range(8):
            sl = s * 8 + j
            pos = int(slot_off[sl])
            rem = int(caps[sl])
            first = True
            while rem > 0:
                t = pos // 128
                p0 = pos % 128          # 0 or 64
                k = min(128 - p0, rem)
                pos += k
                rem -= k
                slist.append((t, p0, k, j, first, rem == 0))
                first = False
        segs.append(slist)

    chunks = []
    s0 = 0
    while s0 < NSTRIP:
        s1 = s0
        cols = 0
        while s1 < NSTRIP and (cols == 0
                               or cols + strip_cols[s1]
                               + strip_cols[s1 + 1] <= 60):
            cols += strip_cols[s1] + strip_cols[s1 + 1]
            s1 += 2
        chunks.append((s0, s1, int(col_base[s0]), int(cols)))
        s0 = s1
    return dict(slot_off=slot_off, strip_cols=strip_cols, col_base=col_base,
                F=F, segs=segs, chunks=chunks)


def _build_program(lay):
    import concourse.bacc as bacc
    import concourse.mybir as mybir
    from concourse.tile import TileContext

    dt = mybir.dt
    Act = mybir.ActivationFunctionType
    Alu = mybir.AluOpType

    F = lay["F"]
    nc = bacc.Bacc("TRN2", target_bir_lowering=False, debug=False)

    dcyx = nc.dram_tensor("dcyx", [P, 2, F], dt.float32, kind="ExternalInput")
    vn = nc.dram_tensor("vn", [P, F], dt.float32, kind="ExternalInput")
    iota = nc.dram_tensor("iota", [P, PW], dt.float32, kind="ExternalInput")
    out = nc.dram_tensor("out", [NSTRIP, PW, 8 * PW], dt.float32,
                         kind="ExternalOutput")

    SQ2 = float(math.sqrt(2.0))

    with TileContext(nc) as tc:
        with (
            tc.tile_pool(name="io", bufs=1) as io,
            tc.tile_pool(name="prof", bufs=1) as prof,
            tc.tile_pool(name="stage", bufs=3) as stage,
            tc.tile_pool(name="psum", bufs=4, space="PSUM") as psum,
        ):
            t_dcyx = io.tile([P, 2, F], dt.float32)
            t_vn = io.tile([P, F], dt.float32)
            t_iota = io.tile([P, PW], dt.float32)
            nc.sync.dma_start(out=t_dcyx[:], in_=dcyx[:])
            nc.sync.dma_start(out=t_vn[:], in_=vn[:])
            nc.sync.dma_start(out=t_iota[:], in_=iota[:])
            t_vnb = io.tile([P, F], dt.bfloat16)
            nc.vector.tensor_copy(out=t_vnb[:], in_=t_vn[:])

            for ci, (s0, s1, c0, gc) in enumerate(lay["chunks"]):
                sl = slice(c0, c0 + gc)
                d = prof.tile([P, 2, gc, PW], dt.float32, tag="d", bufs=3,
                              name=f"d{gc}")
                # fused rd|cd subtract; ~40% of chunks on Pool for balance
                sub_eng = nc.gpsimd if (ci % 8 == 0) else nc.vector
                sub_eng.tensor_tensor(
                    out=d[:],
                    in0=t_iota[:, None, None, :].to_broadcast([P, 2, gc, PW]),
                    in1=t_dcyx[:, :, sl, None].to_broadcast([P, 2, gc, PW]),
                    op=Alu.subtract)
                pr = prof.tile([P, 2, gc, PW], dt.bfloat16, tag="pr", bufs=3,
                               name=f"pr{gc}")
                nc.scalar.activation(out=pr[:], in_=d[:],
                                     func=Act.Derivative_Erf, scale=SQ2)
                colp = prof.tile([P, gc, PW], dt.bfloat16, tag="colp",
                                 bufs=3, name=f"colp{gc}")
                nc.gpsimd.tensor_tensor(
                    out=colp[:], in0=pr[:, 1, :, :],
                    in1=t_vnb[:, sl, None].to_broadcast([P, gc, PW]),
                    op=Alu.mult)

                for s in range(s0, s1):
                    strip = psum.tile([PW, 8 * PW], dt.float32,
                                      tag="strip", name="strip")
                    base = int(lay["col_base"][s]) - c0
                    for (t, p0, k, j, first, last) in lay["segs"][s]:
                        tl = base + t
                        nc.tensor.matmul(
                            out=strip[:, j * PW:(j + 1) * PW],
                            lhsT=pr[p0:p0 + k, 0, tl, :],
                            rhs=colp[p0:p0 + k, tl, :],
                            start=first, stop=last)
                    st = stage.tile([PW, 8 * PW], dt.float32,
                                    tag="st", name="st")
                    nc.scalar.copy(out=st[:], in_=strip[:])
                    nc.sync.dma_start(out=out[s], in_=st[:])
    nc.compile()
    from concourse.bass_interp import get_hw_module
    nc.m = get_hw_module(nc.m)
    return nc


def _host_shard(x, y, values, plan, lay):
    xp = (x.astype(np.float64) + 1.0) * (WIDTH / 2.0)
    yp = (y.astype(np.float64) + 1.0) * (HEIGHT / 2.0)
    xb = np.clip(np.floor(xp).astype(np.int64), 0, WIDTH - 1)
    yb = np.clip(np.floor(yp).astype(np.int64), 0, HEIGHT - 1)
    fx = xp - xb
    fy = yp - yb
    vnorm = (values.astype(np.float64)
             / (2.0 * (1.0 + _Q2 * np.cos(2 * np.pi * fx))
                * (1.0 + _Q2 * np.cos(2 * np.pi * fy))))
    bcx = xb // BLK
    bry = yb // BLK
    gb = bry * GRID + bcx
    core = plan["core_of"][gb]
    slot = plan["slot_of"][gb]
    dcx_all = xp - (bcx * BLK - 2)
    dcy_all = yp - (bry * BLK - 2)

    F = lay["F"]
    # global slot slab base: strip col_base*128 + slot_off
    slab = lay["col_base"][slot // 8] * 128 + lay["slot_off"][slot]

    in_maps = []
    for c in range(N_CORES):
        m = core == c
        ps = slot[m]
        order = np.argsort(ps, kind="stable")
        ps = ps[order]
        counts = np.bincount(ps, minlength=NSLOT)
        if (counts > plan["caps"]).any():
            raise RuntimeError("slot overflow vs caps")
        starts = np.zeros(NSLOT, np.int64)
        np.cumsum(counts[:-1], out=starts[1:])
        idx = np.arange(ps.size) - starts[ps]
        dst = slab[m][order] + idx

        ya = np.full(F * P, 18.0, np.float32)
        xa = np.full(F * P, 18.0, np.float32)
        va = np.zeros(F * P, np.float32)
        ya[dst] = dcy_all[m][order].astype(np.float32)
        xa[dst] = dcx_all[m][order].astype(np.float32)
        va[dst] = vnorm[m][order].astype(np.float32)

        dcyx_a = np.empty((P, 2, F), np.float32)
        dcyx_a[:, 0, :] = ya.reshape(F, P).T
        dcyx_a[:, 1, :] = xa.reshape(F, P).T
        vn_a = np.ascontiguousarray(va.reshape(F, P).T)
        iota_a = np.tile(np.arange(PW, dtype=np.float32), (P, 1))
        in_maps.append({"dcyx": dcyx_a, "vn": vn_a, "iota": iota_a})
    return in_maps


def _assemble(results, plan):
    img = np.zeros((HEIGHT + 4, WIDTH + 4), np.float64)
    for c in range(N_CORES):
        strips = results[c]["out"]          # [NSTRIP, PW, 8*PW]
        for sl in range(NSLOT):
            gb = plan["inv"][c, sl]
            bry, bcx = divmod(int(gb), GRID)
            patch = strips[sl // 8, :, (sl % 8) * PW:(sl % 8 + 1) * PW]
            img[bry * BLK:bry * BLK + PW, bcx * BLK:bcx * BLK + PW] += patch
    return img[2:2 + HEIGHT, 2:2 + WIDTH].astype(np.float32)


def kernel(x, y, values):
    global _COMPILED
    if _COMPILED is None:
        plan = _block_plan(x, y)
        lay = _layout_from_caps(plan["caps"])
        nc = _build_program(lay)
        _COMPILED = (nc, plan, lay)
    nc, plan, lay = _COMPILED
    in_maps = _host_shard(x, y, values, plan, lay)
    from concourse.bass_utils import run_bass_kernel_spmd
    import os
    trace = bool(int(os.environ.get("SPLAT_TRACE", "0")))
    res = run_bass_kernel_spmd(nc, in_maps, list(range(N_CORES)), trace=trace)
    kernel.last_exec_time_ns = res.exec_time_ns
    kernel.last_results = res
    return _assemble(res.results, plan)


kernel.last_exec_time_ns = None



# revision 14
# speedup vs baseline: 1.0017x; 1.0017x over previous
"""Gaussian square-sensor splat on 8 Trainium2 NeuronCores (v4).

Decomposition: the 2048x2048 image is split into 64x64=4096 blocks of
32x32 px.  Each block is assigned to one of 8 cores by COUNT-BALANCED
DEALING: blocks sorted by point count, rank r -> core r%8, slot r//8.
Slot capacities are 32-quantized (caps = ceil(max count in slot/32)*32),
so total column count F ~= 1087 (6% padding).

v4 layout change vs v3: per chunk the work tensors are [P, 36, 2, gc]
(taps MAJOR, columns minor) so every DVE operand has a packed
(stride-1) last dim -> the DVE 2x_1p perf mode engages:
  - d = iota16 - dcq           int16 fixed point (1/256 px), DVE @2x
  - pr = D_ERF(sqrt2/256 * d)  ACT engine, bf16 out
  - colp = pr[:,:,1,:] * vnb   bf16, DVE @2x
  - strip matmuls on PE (accumulate 8 slots per PSUM strip)
  - strip PSUM->SBUF copies on Pool (GPSIMD), DMA out from SBUF
int16 keeps d exact (bf16/fp16 coords would lose ~0.02-0.14 px).
Host precomputes quantized patch offsets and theta-normalized values
vn = v / (2 (1+2q cos 2pi fy)(1+2q cos 2pi fx)).
"""
import math
import sys

sys.path.insert(0, '/opt/trn_rl_repo')

import numpy as np

WIDTH = HEIGHT = 2048
N_POINTS = 1 << 20
N_CORES = 8
BLK = 32
PW = 36
GRID = WIDTH // BLK                     # 64 blocks per side
NBLK = GRID * GRID                      # 4096
NSLOT = NBLK // N_CORES                 # 512 slots per core
NSTRIP = NSLOT // 8                     # 64 psum strips per core
P = 128
GCMAX = 64                              # max columns per chunk
CAPQ = 64                               # slot capacity quantum
                                        # (32 triggers PE 32-row-tile
                                        # transition crashes)
MUL_DVE = 0.14                          # fraction of multiply on DVE

_Q2 = 2.0 * math.exp(-math.pi ** 2 / 2.0)

_COMPILED = None          # (nc, plan, lay)


def _block_plan(x, y):
    """Assign blocks to (core, slot) by count-balanced dealing."""
    xp = (x.astype(np.float64) + 1.0) * (WIDTH / 2.0)
    yp = (y.astype(np.float64) + 1.0) * (HEIGHT / 2.0)
    xb = np.clip(np.floor(xp).astype(np.int64), 0, WIDTH - 1)
    yb = np.clip(np.floor(yp).astype(np.int64), 0, HEIGHT - 1)
    gb = (yb // BLK) * GRID + xb // BLK            # global block id
    counts = np.bincount(gb, minlength=NBLK)
    order = np.argsort(-counts, kind="stable")     # blocks by count desc
    core_of = np.empty(NBLK, np.int64)
    slot_of = np.empty(NBLK, np.int64)
    rank = np.arange(NBLK)
    core_of[order] = rank % N_CORES
    slot_of[order] = rank // N_CORES
    # slot capacity = max count within the slot's 8 blocks, CAPQ-quantized
    slot_max = counts[order].reshape(NSLOT, N_CORES).max(axis=1)
    caps = (np.ceil(slot_max / CAPQ).astype(np.int64) * CAPQ).clip(CAPQ, None)
    # inverse table: (core, slot) -> block id
    inv = np.empty((N_CORES, NSLOT), np.int64)
    inv[core_of[order], slot_of[order]] = order
    return dict(counts=counts, core_of=core_of, slot_of=slot_of,
                caps=caps, inv=inv)


def _layout_from_caps(caps):
    """Slot slab layout: slots packed per strip (8 slots/strip), strips
    padded to whole 128-slot columns.  AP partition bases must be in
    {0,32,64}, so slots within a strip are greedily ordered to avoid
    starts at 96 (mod 128); 32 pad is inserted when unavoidable.
    Matmul segments are split: p0==32 -> k<=32, p0==64 -> k<=64."""
    strip_cols = np.zeros(NSTRIP, np.int64)
    slot_off = np.zeros(NSLOT, np.int64)     # slot offset within strip
    jpos = np.zeros(NSLOT, np.int64)         # position of slot in strip
    col_base = np.zeros(NSTRIP, np.int64)
    segs = []
    for s in range(NSTRIP):
        remaining = list(range(s * 8, s * 8 + 8))
        off = 0
        j = 0
        while remaining:
            # pick a slot whose start keeps (off + cap) starts legal:
            # current start must not be 96 (mod 128)
            if off % 128 == 96:
                off += 32                    # pad to next legal base
            pick = None
            for sl in remaining:
                if (off + int(caps[sl])) % 128 != 96 or len(remaining) == 1:
                    pick = sl
                    break
            if pick is None:
                pick = remaining[0]
            remaining.remove(pick)
            slot_off[pick] = off
            jpos[pick] = j
            off += int(caps[pick])
            j += 1
        strip_cols[s] = (off + 127) // 128
    col_base[1:] = np.cumsum(strip_cols)[:-1]
    F = int(strip_cols.sum())

    for s in range(NSTRIP):
        slist = []
        for sl in range(s * 8, s * 8 + 8):
            pos = int(slot_off[sl])
            rem = int(caps[sl])
            first = True
            j = int(jpos[sl])
            while rem > 0:
                t = pos // 128
                p0 = pos % 128
                assert p0 in (0, 32, 64), f"illegal partition base {p0}"
                k = min(128 - p0, rem)
                if p0 == 32:
                    k = min(k, 32)
                elif p0 == 64:
                    k = min(k, 64)
                pos += k
                rem -= k
                slist.append((t, p0, k, j, first, rem == 0))
                first = False
        segs.append(slist)

    chunks = []
    s0 = 0
    while s0 < NSTRIP:
        s1 = s0
        cols = 0
        while s1 < NSTRIP and cols + strip_cols[s1] <= GCMAX:
            cols += strip_cols[s1]
            s1 += 1
        assert s1 > s0, "single strip exceeds GCMAX"
        chunks.append((s0, s1, int(col_base[s0]), int(cols)))
        s0 = s1
    return dict(slot_off=slot_off, strip_cols=strip_cols, col_base=col_base,
                jpos=jpos, F=F, segs=segs, chunks=chunks)


def _build_program(lay):
    import concourse.bacc as bacc
    import concourse.mybir as mybir
    from concourse.tile import TileContext

    dt = mybir.dt
    Act = mybir.ActivationFunctionType
    Alu = mybir.AluOpType

    F = lay["F"]
    nc = bacc.Bacc("TRN2", target_bir_lowering=False, debug=False)

    dcq = nc.dram_tensor("dcq", [P, 2, F], dt.int16, kind="ExternalInput")
    vnb = nc.dram_tensor("vnb", [P, F], dt.bfloat16, kind="ExternalInput")
    iot = nc.dram_tensor("iot", [P, PW, 2, GCMAX], dt.int16,
                         kind="ExternalInput")
    out = nc.dram_tensor("out", [NSTRIP, PW, 8 * PW], dt.float32,
                         kind="ExternalOutput")

    SC = float(math.sqrt(2.0) / 256.0)

    with TileContext(nc) as tc:
        with (
            tc.tile_pool(name="io", bufs=1) as io,
            tc.tile_pool(name="prof", bufs=1) as prof,
            tc.tile_pool(name="stage", bufs=4) as stage,
            tc.tile_pool(name="psum", bufs=6, space="PSUM") as psum,
        ):
            t_dcq = io.tile([P, 2, F], dt.int16)
            t_vnb = io.tile([P, F], dt.bfloat16)
            t_iot = io.tile([P, PW, 2, GCMAX], dt.int16)
            t_zro = io.tile([P, PW], dt.bfloat16)
            nc.sync.dma_start(out=t_dcq[:], in_=dcq[:])
            nc.sync.dma_start(out=t_vnb[:], in_=vnb[:])
            nc.sync.dma_start(out=t_iot[:], in_=iot[:])
            nc.gpsimd.memset(t_zro[:], 0.0)

            for (s0, s1, c0, gc) in lay["chunks"]:
                sl = slice(c0, c0 + gc)
                d = prof.tile([P, PW, 2, gc], dt.int16, tag="d", bufs=2,
                              name=f"d{gc}")
                nc.vector.tensor_tensor(
                    out=d[:],
                    in0=t_iot[:, :, :, :gc],
                    in1=t_dcq[:, None, :, sl].to_broadcast([P, PW, 2, gc]),
                    op=Alu.subtract)
                pr = prof.tile([P, PW, 2, gc], dt.bfloat16, tag="pr", bufs=3,
                               name=f"pr{gc}")
                nc.scalar.activation(out=pr[:], in_=d[:],
                                     func=Act.Derivative_Erf, scale=SC)
                colp = prof.tile([P, PW, gc], dt.bfloat16, tag="colp",
                                 bufs=3, name=f"colp{gc}")
                # multiply split DVE/Pool for engine balance (by column)
                g1 = max(1, min(gc - 1, int(round(gc * MUL_DVE))))
                nc.vector.tensor_tensor(
                    out=colp[:, :, :g1], in0=pr[:, :, 1, :g1],
                    in1=t_vnb[:, None, c0:c0 + g1].to_broadcast([P, PW, g1]),
                    op=Alu.mult)
                nc.gpsimd.tensor_tensor(
                    out=colp[:, :, g1:], in0=pr[:, :, 1, g1:],
                    in1=t_vnb[:, None, c0 + g1:c0 + gc]
                        .to_broadcast([P, PW, gc - g1]),
                    op=Alu.mult)

                for s in range(s0, s1):
                    strip = psum.tile([PW, 8 * PW], dt.float32,
                                      tag="strip", name="strip")
                    base = int(lay["col_base"][s]) - c0
                    for (t, p0, k, j, first, last) in lay["segs"][s]:
                        tl = base + t
                        if first and p0 != 0:
                            # groups must START at partition base 0 (HW
                            # crash otherwise): open the accumulation
                            # with a k=1 zero matmul at p0=0
                            nc.tensor.matmul(
                                out=strip[:, j * PW:(j + 1) * PW],
                                lhsT=t_zro[0:1, :],
                                rhs=t_zro[0:1, :],
                                start=True, stop=False)
                            first = False
                        nc.tensor.matmul(
                            out=strip[:, j * PW:(j + 1) * PW],
                            lhsT=pr[p0:p0 + k, :, 0, tl],
                            rhs=colp[p0:p0 + k, :, tl],
                            start=first, stop=last)
                    st = stage.tile([PW, 8 * PW], dt.float32,
                                    tag="st", name="st")
                    nc.vector.tensor_copy(out=st[:], in_=strip[:])
                    nc.sync.dma_start(out=out[s], in_=st[:])
    nc.compile()
    from concourse.bass_interp import get_hw_module
    nc.m = get_hw_module(nc.m)
    return nc


def _host_shard(x, y, values, plan, lay):
    from ml_dtypes import bfloat16

    xp = (x.astype(np.float64) + 1.0) * (WIDTH / 2.0)
    yp = (y.astype(np.float64) + 1.0) * (HEIGHT / 2.0)
    xb = np.clip(np.floor(xp).astype(np.int64), 0, WIDTH - 1)
    yb = np.clip(np.floor(yp).astype(np.int64), 0, HEIGHT - 1)
    bcx = xb // BLK
    bry = yb // BLK
    gb = bry * GRID + bcx
    core = plan["core_of"][gb]
    slot = plan["slot_of"][gb]
    dcxq_all = np.round((xp - (bcx * BLK - 2)) * 256).astype(np.int16)
    dcyq_all = np.round((yp - (bry * BLK - 2)) * 256).astype(np.int16)
    fxq = (dcxq_all.astype(np.int64) % 256) / 256.0
    fyq = (dcyq_all.astype(np.int64) % 256) / 256.0
    vnorm = (values.astype(np.float64)
             / (2.0 * (1.0 + _Q2 * np.cos(2 * np.pi * fxq))
                * (1.0 + _Q2 * np.cos(2 * np.pi * fyq))))

    F = lay["F"]
    # global slot slab base: strip col_base*128 + slot_off
    slab = lay["col_base"][slot // 8] * 128 + lay["slot_off"][slot]

    iota_a = np.broadcast_to(
        (np.arange(PW, dtype=np.int16) * 256)[None, :, None, None],
        (P, PW, 2, GCMAX)).copy()

    in_maps = []
    for c in range(N_CORES):
        m = core == c
        ps = slot[m]
        order = np.argsort(ps, kind="stable")
        ps = ps[order]
        counts = np.bincount(ps, minlength=NSLOT)
        if (counts > plan["caps"]).any():
            raise RuntimeError("slot overflow vs caps")
        starts = np.zeros(NSLOT, np.int64)
        np.cumsum(counts[:-1], out=starts[1:])
        idx = np.arange(ps.size) - starts[ps]
        dst = slab[m][order] + idx

        ya = np.full(F * P, 18 * 256, np.int16)
        xa = np.full(F * P, 18 * 256, np.int16)
        va = np.zeros(F * P, np.float64)
        ya[dst] = dcyq_all[m][order]
        xa[dst] = dcxq_all[m][order]
        va[dst] = vnorm[m][order]

        dcq_a = np.empty((P, 2, F), np.int16)
        dcq_a[:, 0, :] = ya.reshape(F, P).T
        dcq_a[:, 1, :] = xa.reshape(F, P).T
        vnb_a = np.ascontiguousarray(va.reshape(F, P).T.astype(bfloat16))
        in_maps.append({"dcq": dcq_a, "vnb": vnb_a, "iot": iota_a})
    return in_maps


def _assemble(results, plan, lay):
    img = np.zeros((HEIGHT + 4, WIDTH + 4), np.float64)
    jpos = lay["jpos"]
    for c in range(N_CORES):
        strips = results[c]["out"]          # [NSTRIP, PW, 8*PW]
        for sl in range(NSLOT):
            gb = plan["inv"][c, sl]
            bry, bcx = divmod(int(gb), GRID)
            j = int(jpos[sl])
            patch = strips[sl // 8, :, j * PW:(j + 1) * PW]
            img[bry * BLK:bry * BLK + PW, bcx * BLK:bcx * BLK + PW] += patch
    return img[2:2 + HEIGHT, 2:2 + WIDTH].astype(np.float32)


def kernel(x, y, values):
    global _COMPILED
    if _COMPILED is None:
        plan = _block_plan(x, y)
        lay = _layout_from_caps(plan["caps"])
        nc = _build_program(lay)
        _COMPILED = (nc, plan, lay)
    nc, plan, lay = _COMPILED
    in_maps = _host_shard(x, y, values, plan, lay)
    from concourse.bass_utils import run_bass_kernel_spmd
    import os
    trace = bool(int(os.environ.get("SPLAT_TRACE", "0")))
    res = run_bass_kernel_spmd(nc, in_maps, list(range(N_CORES)), trace=trace)
    kernel.last_exec_time_ns = res.exec_time_ns
    kernel.last_results = res
    return _assemble(res.results, plan, lay)


kernel.last_exec_time_ns = None


# revision 15
# speedup vs baseline: 1.1797x; 1.1777x over previous
"""Gaussian square-sensor splat on 8 Trainium2 NeuronCores (v5).

Decomposition: the 2048x2048 image is split into 64x64=4096 blocks of
32x32 px.  Each block is assigned to one of 8 cores by COUNT-BALANCED
DEALING: blocks sorted by point count, rank r -> core r%8, slot r//8.
Slot capacities are 64-quantized; 8 slots pack into each PSUM strip.

v5 layout: per chunk the work tensors are [P, 2*gc, 18, 2] with the
taps (36 = 18*2) CONTIGUOUS per (dim, column) slab:
  - matmul operands pr/colp slabs [k, 36] are contiguous (fast PE)
  - every DVE operand still ends in a packed [1,2] dim, so the DVE
    2x_1p perf mode engages: broadcasts (dc over taps, vn over taps)
    are expressed via PAIR-DUPLICATED host inputs dcq2/vnb2 whose AP
    last dim is [stride 1, count 2]
  - d = iota16 - dcq2          int16 fixed point (1/256 px), DVE @2x
  - pr = D_ERF(sqrt2/256 * d)  ACT engine, bf16, packed read/write
  - colp = pr[col] * vnb2      bf16 @2x, split DVE/Pool for balance
  - strip matmuls on PE accumulate 8 slots per [36, 288] PSUM strip;
    accumulation groups are reordered to START at partition base 0
    (HW crash otherwise); rare all-offset slots open with a k=1 zero
    matmul
  - strip PSUM->SBUF copies on DVE, DMA out from SBUF
int16 keeps d exact (bf16/fp16 coords would lose 0.02-0.14 px).
Host precomputes quantized patch offsets and theta-normalized values
vn = v / (2 (1+2q cos 2pi fy)(1+2q cos 2pi fx)).
"""
import math
import sys

sys.path.insert(0, '/opt/trn_rl_repo')

import numpy as np

WIDTH = HEIGHT = 2048
N_POINTS = 1 << 20
N_CORES = 8
BLK = 32
PW = 36
GRID = WIDTH // BLK                     # 64 blocks per side
NBLK = GRID * GRID                      # 4096
NSLOT = NBLK // N_CORES                 # 512 slots per core
NSTRIP = NSLOT // 8                     # 64 psum strips per core
P = 128
GCMAX = 64                              # max columns per chunk
CAPQ = 64                               # slot capacity quantum
                                        # (32 triggers PE 32-row-tile
                                        # transition crashes)
MUL_DVE = 0.14                          # fraction of multiply on DVE

_Q2 = 2.0 * math.exp(-math.pi ** 2 / 2.0)

_COMPILED = None          # (nc, plan, lay)


def _block_plan(x, y):
    """Assign blocks to (core, slot) by count-balanced dealing."""
    xp = (x.astype(np.float64) + 1.0) * (WIDTH / 2.0)
    yp = (y.astype(np.float64) + 1.0) * (HEIGHT / 2.0)
    xb = np.clip(np.floor(xp).astype(np.int64), 0, WIDTH - 1)
    yb = np.clip(np.floor(yp).astype(np.int64), 0, HEIGHT - 1)
    gb = (yb // BLK) * GRID + xb // BLK            # global block id
    counts = np.bincount(gb, minlength=NBLK)
    order = np.argsort(-counts, kind="stable")     # blocks by count desc
    core_of = np.empty(NBLK, np.int64)
    slot_of = np.empty(NBLK, np.int64)
    rank = np.arange(NBLK)
    core_of[order] = rank % N_CORES
    slot_of[order] = rank // N_CORES
    # slot capacity = max count within the slot's 8 blocks, CAPQ-quantized
    slot_max = counts[order].reshape(NSLOT, N_CORES).max(axis=1)
    caps = (np.ceil(slot_max / CAPQ).astype(np.int64) * CAPQ).clip(CAPQ, None)
    # inverse table: (core, slot) -> block id
    inv = np.empty((N_CORES, NSLOT), np.int64)
    inv[core_of[order], slot_of[order]] = order
    return dict(counts=counts, core_of=core_of, slot_of=slot_of,
                caps=caps, inv=inv)


def _layout_from_caps(caps):
    """Slot slab layout: slots packed per strip (8 slots/strip), strips
    padded to whole 128-slot columns.  Matmul segments: p0==64 -> k<=64.
    Each slot's segment list is rotated so a p0==0 segment (if any)
    comes first and carries start=True; slots with no p0==0 segment are
    marked need_zero (opened by a k=1 zero matmul)."""
    strip_cols = np.zeros(NSTRIP, np.int64)
    slot_off = np.zeros(NSLOT, np.int64)     # slot offset within strip
    jpos = np.zeros(NSLOT, np.int64)         # position of slot in strip
    col_base = np.zeros(NSTRIP, np.int64)
    segs = []
    for s in range(NSTRIP):
        off = 0
        for j in range(8):
            sl = s * 8 + j
            slot_off[sl] = off
            jpos[sl] = j
            off += int(caps[sl])
        strip_cols[s] = (off + 127) // 128
    col_base[1:] = np.cumsum(strip_cols)[:-1]
    F = int(strip_cols.sum())

    nzero = 0
    for s in range(NSTRIP):
        slist = []
        for sl in range(s * 8, s * 8 + 8):
            pos = int(slot_off[sl])
            rem = int(caps[sl])
            j = int(jpos[sl])
            parts = []
            while rem > 0:
                t = pos // 128
                p0 = pos % 128
                assert p0 in (0, 64), f"illegal partition base {p0}"
                k = min(128 - p0, rem)
                if p0 == 64:
                    k = min(k, 64)
                pos += k
                rem -= k
                parts.append((t, p0, k))
            # rotate a p0==0 part to the front (group must start at
            # partition base 0)
            i0 = next((i for i, e in enumerate(parts) if e[1] == 0), None)
            if i0 is None:
                nzero += 1
                need_zero = True
            else:
                parts = parts[i0:i0 + 1] + parts[:i0] + parts[i0 + 1:]
                need_zero = False
            n = len(parts)
            for i, (t, p0, k) in enumerate(parts):
                first = (i == 0) and not need_zero
                slist.append((t, p0, k, j, first, i == n - 1,
                              need_zero and i == 0))
        segs.append(slist)

    chunks = []
    s0 = 0
    while s0 < NSTRIP:
        s1 = s0
        cols = 0
        while s1 < NSTRIP and cols + strip_cols[s1] <= GCMAX:
            cols += strip_cols[s1]
            s1 += 1
        assert s1 > s0, "single strip exceeds GCMAX"
        chunks.append((s0, s1, int(col_base[s0]), int(cols)))
        s0 = s1
    return dict(slot_off=slot_off, strip_cols=strip_cols, col_base=col_base,
                jpos=jpos, F=F, segs=segs, chunks=chunks, nzero=nzero)


def _build_program(lay):
    import concourse.bacc as bacc
    import concourse.mybir as mybir
    from concourse.tile import TileContext

    dt = mybir.dt
    Act = mybir.ActivationFunctionType
    Alu = mybir.AluOpType

    F = lay["F"]
    nc = bacc.Bacc("TRN2", target_bir_lowering=False, debug=False)

    # dcq2: per chunk, [dcy cols | dcx cols] pair-duplicated -> [P, 2F, 2]
    dcq2 = nc.dram_tensor("dcq2", [P, 2 * F, 2], dt.int16,
                          kind="ExternalInput")
    # vnb2: F columns pair-duplicated -> [P, F, 2]
    vnb2 = nc.dram_tensor("vnb2", [P, F, 2], dt.bfloat16,
                          kind="ExternalInput")
    iot = nc.dram_tensor("iot", [P, 18, 2], dt.int16, kind="ExternalInput")
    out = nc.dram_tensor("out", [NSTRIP, PW, 8 * PW], dt.float32,
                         kind="ExternalOutput")

    SC = float(math.sqrt(2.0) / 256.0)

    with TileContext(nc) as tc:
        with (
            tc.tile_pool(name="io", bufs=1) as io,
            tc.tile_pool(name="prof", bufs=1) as prof,
            tc.tile_pool(name="stage", bufs=4) as stage,
            tc.tile_pool(name="psum", bufs=6, space="PSUM") as psum,
        ):
            t_dcq2 = io.tile([P, 2 * F, 2], dt.int16)
            t_vnb2 = io.tile([P, F, 2], dt.bfloat16)
            t_iot = io.tile([P, 18, 2], dt.int16)
            t_zro = io.tile([P, PW], dt.bfloat16)
            nc.sync.dma_start(out=t_dcq2[:], in_=dcq2[:])
            nc.sync.dma_start(out=t_vnb2[:], in_=vnb2[:])
            nc.sync.dma_start(out=t_iot[:], in_=iot[:])
            nc.gpsimd.memset(t_zro[:], 0.0)

            co2 = 0                     # running column offset into dcq2
            for (s0, s1, c0, gc) in lay["chunks"]:
                d = prof.tile([P, 2 * gc, 18, 2], dt.int16, tag="d", bufs=2,
                              name=f"d{gc}")
                nc.vector.tensor_tensor(
                    out=d[:],
                    in0=t_iot[:, None, :, :].to_broadcast([P, 2 * gc, 18, 2]),
                    in1=t_dcq2[:, co2:co2 + 2 * gc, None, :]
                        .to_broadcast([P, 2 * gc, 18, 2]),
                    op=Alu.subtract)
                pr = prof.tile([P, 2 * gc, 18, 2], dt.bfloat16, tag="pr",
                               bufs=3, name=f"pr{gc}")
                nc.scalar.activation(out=pr[:], in_=d[:],
                                     func=Act.Derivative_Erf, scale=SC)
                colp = prof.tile([P, gc, 18, 2], dt.bfloat16, tag="colp",
                                 bufs=3, name=f"colp{gc}")
                # multiply split DVE/Pool for engine balance (by column)
                g1 = max(1, min(gc, int(round(gc * MUL_DVE))))
                nc.vector.tensor_tensor(
                    out=colp[:, :g1], in0=pr[:, gc:gc + g1],
                    in1=t_vnb2[:, c0:c0 + g1, None, :]
                        .to_broadcast([P, g1, 18, 2]),
                    op=Alu.mult)
                if g1 < gc:
                    nc.gpsimd.tensor_tensor(
                        out=colp[:, g1:], in0=pr[:, gc + g1:2 * gc],
                        in1=t_vnb2[:, c0 + g1:c0 + gc, None, :]
                            .to_broadcast([P, gc - g1, 18, 2]),
                        op=Alu.mult)

                for s in range(s0, s1):
                    strip = psum.tile([PW, 8 * PW], dt.float32,
                                      tag="strip", name="strip")
                    base = int(lay["col_base"][s]) - c0
                    for (t, p0, k, j, first, last, zopen) in lay["segs"][s]:
                        tl = base + t
                        if zopen:
                            nc.tensor.matmul(
                                out=strip[:, j * PW:(j + 1) * PW],
                                lhsT=t_zro[0:1, :],
                                rhs=t_zro[0:1, :],
                                start=True, stop=False)
                        nc.tensor.matmul(
                            out=strip[:, j * PW:(j + 1) * PW],
                            lhsT=pr[p0:p0 + k, tl],
                            rhs=colp[p0:p0 + k, tl],
                            start=first, stop=last)
                    st = stage.tile([PW, 8 * PW], dt.float32,
                                    tag="st", name="st")
                    nc.vector.tensor_copy(out=st[:], in_=strip[:])
                    nc.sync.dma_start(out=out[s], in_=st[:])
                co2 += 2 * gc
    nc.compile()
    from concourse.bass_interp import get_hw_module
    nc.m = get_hw_module(nc.m)
    return nc


def _host_shard(x, y, values, plan, lay):
    from ml_dtypes import bfloat16

    xp = (x.astype(np.float64) + 1.0) * (WIDTH / 2.0)
    yp = (y.astype(np.float64) + 1.0) * (HEIGHT / 2.0)
    xb = np.clip(np.floor(xp).astype(np.int64), 0, WIDTH - 1)
    yb = np.clip(np.floor(yp).astype(np.int64), 0, HEIGHT - 1)
    bcx = xb // BLK
    bry = yb // BLK
    gb = bry * GRID + bcx
    core = plan["core_of"][gb]
    slot = plan["slot_of"][gb]
    dcxq_all = np.round((xp - (bcx * BLK - 2)) * 256).astype(np.int16)
    dcyq_all = np.round((yp - (bry * BLK - 2)) * 256).astype(np.int16)
    fxq = (dcxq_all.astype(np.int64) % 256) / 256.0
    fyq = (dcyq_all.astype(np.int64) % 256) / 256.0
    vnorm = (values.astype(np.float64)
             / (2.0 * (1.0 + _Q2 * np.cos(2 * np.pi * fxq))
                * (1.0 + _Q2 * np.cos(2 * np.pi * fyq))))

    F = lay["F"]
    # global slot slab base: strip col_base*128 + slot_off
    slab = lay["col_base"][slot // 8] * 128 + lay["slot_off"][slot]

    iota_a = np.broadcast_to((np.arange(PW, dtype=np.int16) * 256)[None, :],
                             (P, PW)).reshape(P, 18, 2).copy()

    in_maps = []
    for c in range(N_CORES):
        m = core == c
        ps = slot[m]
        order = np.argsort(ps, kind="stable")
        ps = ps[order]
        counts = np.bincount(ps, minlength=NSLOT)
        if (counts > plan["caps"]).any():
            raise RuntimeError("slot overflow vs caps")
        starts = np.zeros(NSLOT, np.int64)
        np.cumsum(counts[:-1], out=starts[1:])
        idx = np.arange(ps.size) - starts[ps]
        dst = slab[m][order] + idx

        ya = np.full(F * P, 18 * 256, np.int16)
        xa = np.full(F * P, 18 * 256, np.int16)
        va = np.zeros(F * P, np.float64)
        ya[dst] = dcyq_all[m][order]
        xa[dst] = dcxq_all[m][order]
        va[dst] = vnorm[m][order]

        yaT = ya.reshape(F, P).T            # [P, F]
        xaT = xa.reshape(F, P).T
        vaT = va.reshape(F, P).T

        # dcq2: per chunk [dcy cols | dcx cols], each value duplicated
        dcq2_a = np.empty((P, 2 * F, 2), np.int16)
        off = 0
        for (_, _, c0, gc) in lay["chunks"]:
            dcq2_a[:, off:off + gc, 0] = yaT[:, c0:c0 + gc]
            dcq2_a[:, off + gc:off + 2 * gc, 0] = xaT[:, c0:c0 + gc]
            off += 2 * gc
        dcq2_a[:, :, 1] = dcq2_a[:, :, 0]

        vnb2_a = np.empty((P, F, 2), bfloat16)
        vnb2_a[:, :, 0] = vaT.astype(bfloat16)
        vnb2_a[:, :, 1] = vnb2_a[:, :, 0]
        in_maps.append({"dcq2": dcq2_a, "vnb2": np.ascontiguousarray(vnb2_a),
                        "iot": iota_a})
    return in_maps


def _assemble(results, plan, lay):
    img = np.zeros((HEIGHT + 4, WIDTH + 4), np.float64)
    jpos = lay["jpos"]
    for c in range(N_CORES):
        strips = results[c]["out"]          # [NSTRIP, PW, 8*PW]
        for sl in range(NSLOT):
            gb = plan["inv"][c, sl]
            bry, bcx = divmod(int(gb), GRID)
            j = int(jpos[sl])
            patch = strips[sl // 8, :, j * PW:(j + 1) * PW]
            img[bry * BLK:bry * BLK + PW, bcx * BLK:bcx * BLK + PW] += patch
    return img[2:2 + HEIGHT, 2:2 + WIDTH].astype(np.float32)


def kernel(x, y, values):
    global _COMPILED
    if _COMPILED is None:
        plan = _block_plan(x, y)
        lay = _layout_from_caps(plan["caps"])
        nc = _build_program(lay)
        _COMPILED = (nc, plan, lay)
    nc, plan, lay = _COMPILED
    in_maps = _host_shard(x, y, values, plan, lay)
    from concourse.bass_utils import run_bass_kernel_spmd
    import os
    trace = bool(int(os.environ.get("SPLAT_TRACE", "0")))
    res = run_bass_kernel_spmd(nc, in_maps, list(range(N_CORES)), trace=trace)
    kernel.last_exec_time_ns = res.exec_time_ns
    kernel.last_results = res
    return _assemble(res.results, plan, lay)


kernel.last_exec_time_ns = None


# revision 17
# speedup vs baseline: 1.2847x; 1.0890x over previous
"""Gaussian square-sensor splat on 8 Trainium2 NeuronCores (v5).

Decomposition: the 2048x2048 image is split into 64x64=4096 blocks of
32x32 px.  Each block is assigned to one of 8 cores by COUNT-BALANCED
DEALING: blocks sorted by point count, rank r -> core r%8, slot r//8.
Slot capacities are 64-quantized; 8 slots pack into each PSUM strip.

v5 layout: per chunk the work tensors are [P, 2*gc, 18, 2] with the
taps (36 = 18*2) CONTIGUOUS per (dim, column) slab:
  - matmul operands pr/colp slabs [k, 36] are contiguous (fast PE)
  - every DVE operand still ends in a packed [1,2] dim, so the DVE
    2x_1p perf mode engages: broadcasts (dc over taps, vn over taps)
    are expressed via PAIR-DUPLICATED host inputs dcq2/vnb2 whose AP
    last dim is [stride 1, count 2]
  - d = iota16 - dcq2          int16 fixed point (1/256 px), DVE @2x
  - pr = D_ERF(sqrt2/256 * d)  ACT engine, bf16, packed read/write
  - colp = pr[col] * vnb2      bf16 @2x, split DVE/Pool for balance
  - strip matmuls on PE accumulate 8 slots per [36, 288] PSUM strip;
    accumulation groups are reordered to START at partition base 0
    (HW crash otherwise); rare all-offset slots open with a k=1 zero
    matmul
  - strip PSUM->SBUF copies on DVE, DMA out from SBUF
int16 keeps d exact (bf16/fp16 coords would lose 0.02-0.14 px).
Host precomputes quantized patch offsets and theta-normalized values
vn = v / (2 (1+2q cos 2pi fy)(1+2q cos 2pi fx)).
"""
import math
import sys

sys.path.insert(0, '/opt/trn_rl_repo')

import numpy as np

WIDTH = HEIGHT = 2048
N_POINTS = 1 << 20
N_CORES = 8
BLK = 32
PW = 36
GRID = WIDTH // BLK                     # 64 blocks per side
NBLK = GRID * GRID                      # 4096
NSLOT = NBLK // N_CORES                 # 512 slots per core
NSTRIP = NSLOT // 8                     # 64 psum strips per core
P = 128
GCMAX = 64                              # max columns per chunk
CAPQ = 64                               # slot capacity quantum
                                        # (32 triggers PE 32-row-tile
                                        # transition crashes)
MUL_DVE = 0.0                           # fraction of multiply on DVE

_Q2 = 2.0 * math.exp(-math.pi ** 2 / 2.0)

_COMPILED = None          # (nc, plan, lay)


def _block_plan(x, y):
    """Assign blocks to (core, slot) by count-balanced dealing."""
    xp = (x.astype(np.float64) + 1.0) * (WIDTH / 2.0)
    yp = (y.astype(np.float64) + 1.0) * (HEIGHT / 2.0)
    xb = np.clip(np.floor(xp).astype(np.int64), 0, WIDTH - 1)
    yb = np.clip(np.floor(yp).astype(np.int64), 0, HEIGHT - 1)
    gb = (yb // BLK) * GRID + xb // BLK            # global block id
    counts = np.bincount(gb, minlength=NBLK)
    order = np.argsort(-counts, kind="stable")     # blocks by count desc
    core_of = np.empty(NBLK, np.int64)
    slot_of = np.empty(NBLK, np.int64)
    rank = np.arange(NBLK)
    core_of[order] = rank % N_CORES
    slot_of[order] = rank // N_CORES
    # slot capacity = max count within the slot's 8 blocks, CAPQ-quantized
    slot_max = counts[order].reshape(NSLOT, N_CORES).max(axis=1)
    caps = (np.ceil(slot_max / CAPQ).astype(np.int64) * CAPQ).clip(CAPQ, None)
    # inverse table: (core, slot) -> block id
    inv = np.empty((N_CORES, NSLOT), np.int64)
    inv[core_of[order], slot_of[order]] = order
    return dict(counts=counts, core_of=core_of, slot_of=slot_of,
                caps=caps, inv=inv)


def _layout_from_caps(caps):
    """Slot slab layout: slots packed per strip (8 slots/strip), strips
    padded to whole 128-slot columns.  Matmul segments: p0==64 -> k<=64.
    Each slot's segment list is rotated so a p0==0 segment (if any)
    comes first and carries start=True; slots with no p0==0 segment are
    marked need_zero (opened by a k=1 zero matmul)."""
    strip_cols = np.zeros(NSTRIP, np.int64)
    slot_off = np.zeros(NSLOT, np.int64)     # slot offset within strip
    jpos = np.zeros(NSLOT, np.int64)         # position of slot in strip
    col_base = np.zeros(NSTRIP, np.int64)
    segs = []
    for s in range(NSTRIP):
        off = 0
        for j in range(8):
            sl = s * 8 + j
            slot_off[sl] = off
            jpos[sl] = j
            off += int(caps[sl])
        strip_cols[s] = (off + 127) // 128
    col_base[1:] = np.cumsum(strip_cols)[:-1]
    F = int(strip_cols.sum())

    nzero = 0
    for s in range(NSTRIP):
        slist = []
        for sl in range(s * 8, s * 8 + 8):
            pos = int(slot_off[sl])
            rem = int(caps[sl])
            j = int(jpos[sl])
            parts = []
            while rem > 0:
                t = pos // 128
                p0 = pos % 128
                assert p0 in (0, 64), f"illegal partition base {p0}"
                k = min(128 - p0, rem)
                if p0 == 64:
                    k = min(k, 64)
                pos += k
                rem -= k
                parts.append((t, p0, k))
            # rotate a p0==0 part to the front (group must start at
            # partition base 0)
            i0 = next((i for i, e in enumerate(parts) if e[1] == 0), None)
            if i0 is None:
                nzero += 1
                need_zero = True
            else:
                parts = parts[i0:i0 + 1] + parts[:i0] + parts[i0 + 1:]
                need_zero = False
            n = len(parts)
            for i, (t, p0, k) in enumerate(parts):
                first = (i == 0) and not need_zero
                slist.append((t, p0, k, j, first, i == n - 1,
                              need_zero and i == 0))
        segs.append(slist)

    chunks = []
    s0 = 0
    while s0 < NSTRIP:
        s1 = s0
        cols = 0
        while s1 < NSTRIP and cols + strip_cols[s1] <= GCMAX:
            cols += strip_cols[s1]
            s1 += 1
        assert s1 > s0, "single strip exceeds GCMAX"
        chunks.append((s0, s1, int(col_base[s0]), int(cols)))
        s0 = s1
    return dict(slot_off=slot_off, strip_cols=strip_cols, col_base=col_base,
                jpos=jpos, F=F, segs=segs, chunks=chunks, nzero=nzero)


def _build_program(lay):
    import concourse.bacc as bacc
    import concourse.mybir as mybir
    from concourse.tile import TileContext

    dt = mybir.dt
    Act = mybir.ActivationFunctionType
    Alu = mybir.AluOpType

    F = lay["F"]
    nc = bacc.Bacc("TRN2", target_bir_lowering=False, debug=False)

    # dcq2: per chunk, [dcy cols | dcx cols] pair-duplicated -> [P, 2F, 2]
    dcq2 = nc.dram_tensor("dcq2", [P, 2 * F, 2], dt.int16,
                          kind="ExternalInput")
    # vnb2: F columns pair-duplicated -> [P, F, 2]
    vnb2 = nc.dram_tensor("vnb2", [P, F, 2], dt.bfloat16,
                          kind="ExternalInput")
    iot = nc.dram_tensor("iot", [P, 18, 2], dt.int16, kind="ExternalInput")
    out = nc.dram_tensor("out", [NSTRIP, PW, 8 * PW], dt.float32,
                         kind="ExternalOutput")

    SC = float(math.sqrt(2.0) / 256.0)

    with TileContext(nc) as tc:
        with (
            tc.tile_pool(name="io", bufs=1) as io,
            tc.tile_pool(name="prof", bufs=1) as prof,
            tc.tile_pool(name="stage", bufs=4) as stage,
            tc.tile_pool(name="psum", bufs=6, space="PSUM") as psum,
        ):
            t_dcq2 = io.tile([P, 2 * F, 2], dt.int16)
            t_vnb2 = io.tile([P, F, 2], dt.bfloat16)
            t_iot = io.tile([P, 18, 2], dt.int16)
            t_zro = io.tile([P, PW], dt.bfloat16)
            nc.sync.dma_start(out=t_dcq2[:], in_=dcq2[:])
            nc.sync.dma_start(out=t_vnb2[:], in_=vnb2[:])
            nc.sync.dma_start(out=t_iot[:], in_=iot[:])
            nc.gpsimd.memset(t_zro[:], 0.0)

            def emit_mm(ck):
                """Matmuls + strip copies + DMAs for a chunk's strips."""
                (s0, s1, c0, gc), pr, colp = ck
                for s in range(s0, s1):
                    strip = psum.tile([PW, 8 * PW], dt.float32,
                                      tag="strip", name="strip")
                    base = int(lay["col_base"][s]) - c0
                    for (t, p0, k, j, first, last, zopen) in lay["segs"][s]:
                        tl = base + t
                        if zopen:
                            nc.tensor.matmul(
                                out=strip[:, j * PW:(j + 1) * PW],
                                lhsT=t_zro[0:1, :],
                                rhs=t_zro[0:1, :],
                                start=True, stop=False)
                        nc.tensor.matmul(
                            out=strip[:, j * PW:(j + 1) * PW],
                            lhsT=pr[p0:p0 + k, tl],
                            rhs=colp[p0:p0 + k, tl],
                            start=first, stop=last)
                    st = stage.tile([PW, 8 * PW], dt.float32,
                                    tag="st", name="st")
                    nc.vector.tensor_copy(out=st[:], in_=strip[:])
                    nc.sync.dma_start(out=out[s], in_=st[:])

            co2 = 0                     # running column offset into dcq2
            prev = None                 # software pipeline: mm lags 1 chunk
            for (s0, s1, c0, gc) in lay["chunks"]:
                d = prof.tile([P, 2 * gc, 18, 2], dt.int16, tag="d", bufs=2,
                              name=f"d{gc}")
                nc.vector.tensor_tensor(
                    out=d[:],
                    in0=t_iot[:, None, :, :].to_broadcast([P, 2 * gc, 18, 2]),
                    in1=t_dcq2[:, co2:co2 + 2 * gc, None, :]
                        .to_broadcast([P, 2 * gc, 18, 2]),
                    op=Alu.subtract)
                pr = prof.tile([P, 2 * gc, 18, 2], dt.bfloat16, tag="pr",
                               bufs=3, name=f"pr{gc}")
                nc.scalar.activation(out=pr[:], in_=d[:],
                                     func=Act.Derivative_Erf, scale=SC)
                colp = prof.tile([P, gc, 18, 2], dt.bfloat16, tag="colp",
                                 bufs=3, name=f"colp{gc}")
                # multiply split DVE/Pool for engine balance (by column)
                g1 = max(0, min(gc, int(round(gc * MUL_DVE))))
                if g1 > 0:
                    nc.vector.tensor_tensor(
                        out=colp[:, :g1], in0=pr[:, gc:gc + g1],
                        in1=t_vnb2[:, c0:c0 + g1, None, :]
                            .to_broadcast([P, g1, 18, 2]),
                        op=Alu.mult)
                if g1 < gc:
                    nc.gpsimd.tensor_tensor(
                        out=colp[:, g1:], in0=pr[:, gc + g1:2 * gc],
                        in1=t_vnb2[:, c0 + g1:c0 + gc, None, :]
                            .to_broadcast([P, gc - g1, 18, 2]),
                        op=Alu.mult)

                if prev is not None:
                    emit_mm(prev)
                prev = ((s0, s1, c0, gc), pr, colp)
                co2 += 2 * gc
            emit_mm(prev)
    nc.compile()
    from concourse.bass_interp import get_hw_module
    nc.m = get_hw_module(nc.m)
    return nc


def _host_shard(x, y, values, plan, lay):
    from ml_dtypes import bfloat16

    xp = (x.astype(np.float64) + 1.0) * (WIDTH / 2.0)
    yp = (y.astype(np.float64) + 1.0) * (HEIGHT / 2.0)
    xb = np.clip(np.floor(xp).astype(np.int64), 0, WIDTH - 1)
    yb = np.clip(np.floor(yp).astype(np.int64), 0, HEIGHT - 1)
    bcx = xb // BLK
    bry = yb // BLK
    gb = bry * GRID + bcx
    core = plan["core_of"][gb]
    slot = plan["slot_of"][gb]
    dcxq_all = np.round((xp - (bcx * BLK - 2)) * 256).astype(np.int16)
    dcyq_all = np.round((yp - (bry * BLK - 2)) * 256).astype(np.int16)
    fxq = (dcxq_all.astype(np.int64) % 256) / 256.0
    fyq = (dcyq_all.astype(np.int64) % 256) / 256.0
    vnorm = (values.astype(np.float64)
             / (2.0 * (1.0 + _Q2 * np.cos(2 * np.pi * fxq))
                * (1.0 + _Q2 * np.cos(2 * np.pi * fyq))))

    F = lay["F"]
    # global slot slab base: strip col_base*128 + slot_off
    slab = lay["col_base"][slot // 8] * 128 + lay["slot_off"][slot]

    iota_a = np.broadcast_to((np.arange(PW, dtype=np.int16) * 256)[None, :],
                             (P, PW)).reshape(P, 18, 2).copy()

    in_maps = []
    for c in range(N_CORES):
        m = core == c
        ps = slot[m]
        order = np.argsort(ps, kind="stable")
        ps = ps[order]
        counts = np.bincount(ps, minlength=NSLOT)
        if (counts > plan["caps"]).any():
            raise RuntimeError("slot overflow vs caps")
        starts = np.zeros(NSLOT, np.int64)
        np.cumsum(counts[:-1], out=starts[1:])
        idx = np.arange(ps.size) - starts[ps]
        dst = slab[m][order] + idx

        ya = np.full(F * P, 18 * 256, np.int16)
        xa = np.full(F * P, 18 * 256, np.int16)
        va = np.zeros(F * P, np.float64)
        ya[dst] = dcyq_all[m][order]
        xa[dst] = dcxq_all[m][order]
        va[dst] = vnorm[m][order]

        yaT = ya.reshape(F, P).T            # [P, F]
        xaT = xa.reshape(F, P).T
        vaT = va.reshape(F, P).T

        # dcq2: per chunk [dcy cols | dcx cols], each value duplicated
        dcq2_a = np.empty((P, 2 * F, 2), np.int16)
        off = 0
        for (_, _, c0, gc) in lay["chunks"]:
            dcq2_a[:, off:off + gc, 0] = yaT[:, c0:c0 + gc]
            dcq2_a[:, off + gc:off + 2 * gc, 0] = xaT[:, c0:c0 + gc]
            off += 2 * gc
        dcq2_a[:, :, 1] = dcq2_a[:, :, 0]

        vnb2_a = np.empty((P, F, 2), bfloat16)
        vnb2_a[:, :, 0] = vaT.astype(bfloat16)
        vnb2_a[:, :, 1] = vnb2_a[:, :, 0]
        in_maps.append({"dcq2": dcq2_a, "vnb2": np.ascontiguousarray(vnb2_a),
                        "iot": iota_a})
    return in_maps


def _assemble(results, plan, lay):
    img = np.zeros((HEIGHT + 4, WIDTH + 4), np.float64)
    jpos = lay["jpos"]
    for c in range(N_CORES):
        strips = results[c]["out"]          # [NSTRIP, PW, 8*PW]
        for sl in range(NSLOT):
            gb = plan["inv"][c, sl]
            bry, bcx = divmod(int(gb), GRID)
            j = int(jpos[sl])
            patch = strips[sl // 8, :, j * PW:(j + 1) * PW]
            img[bry * BLK:bry * BLK + PW, bcx * BLK:bcx * BLK + PW] += patch
    return img[2:2 + HEIGHT, 2:2 + WIDTH].astype(np.float32)


def kernel(x, y, values):
    global _COMPILED
    if _COMPILED is None:
        plan = _block_plan(x, y)
        lay = _layout_from_caps(plan["caps"])
        nc = _build_program(lay)
        _COMPILED = (nc, plan, lay)
    nc, plan, lay = _COMPILED
    in_maps = _host_shard(x, y, values, plan, lay)
    from concourse.bass_utils import run_bass_kernel_spmd
    import os
    trace = bool(int(os.environ.get("SPLAT_TRACE", "0")))
    res = run_bass_kernel_spmd(nc, in_maps, list(range(N_CORES)), trace=trace)
    kernel.last_exec_time_ns = res.exec_time_ns
    kernel.last_results = res
    return _assemble(res.results, plan, lay)


kernel.last_exec_time_ns = None


# revision 20
# speedup vs baseline: 1.3356x; 1.0396x over previous
"""Gaussian square-sensor splat on 8 Trainium2 NeuronCores (v5).

Decomposition: the 2048x2048 image is split into 64x64=4096 blocks of
32x32 px.  Each block is assigned to one of 8 cores by COUNT-BALANCED
DEALING: blocks sorted by point count, rank r -> core r%8, slot r//8.
Slot capacities are 64-quantized; 8 slots pack into each PSUM strip.

v5 layout: per chunk the work tensors are [P, 2*gc, 18, 2] with the
taps (36 = 18*2) CONTIGUOUS per (dim, column) slab:
  - matmul operands pr/colp slabs [k, 36] are contiguous (fast PE)
  - every DVE operand still ends in a packed [1,2] dim, so the DVE
    2x_1p perf mode engages: broadcasts (dc over taps, vn over taps)
    are expressed via PAIR-DUPLICATED host inputs dcq2/vnb2 whose AP
    last dim is [stride 1, count 2]
  - d = iota16 - dcq2          int16 fixed point (1/256 px), DVE @2x
  - pr = D_ERF(sqrt2/256 * d)  ACT engine, bf16, packed read/write
  - colp = pr[col] * vnb2      bf16 @2x, split DVE/Pool for balance
  - strip matmuls on PE accumulate 8 slots per [36, 288] PSUM strip;
    accumulation groups are reordered to START at partition base 0
    (HW crash otherwise); rare all-offset slots open with a k=1 zero
    matmul
  - strip PSUM->SBUF copies on DVE, DMA out from SBUF
int16 keeps d exact (bf16/fp16 coords would lose 0.02-0.14 px).
Host precomputes quantized patch offsets and theta-normalized values
vn = v / (2 (1+2q cos 2pi fy)(1+2q cos 2pi fx)).
"""
import math
import sys

sys.path.insert(0, '/opt/trn_rl_repo')

import numpy as np

WIDTH = HEIGHT = 2048
N_POINTS = 1 << 20
N_CORES = 8
BLK = 32
PW = 36
GRID = WIDTH // BLK                     # 64 blocks per side
NBLK = GRID * GRID                      # 4096
NSLOT = NBLK // N_CORES                 # 512 slots per core
NSTRIP = NSLOT // 8                     # 64 psum strips per core
P = 128
GCMAX = 64                              # max columns per chunk
CAPQ = 64                               # slot capacity quantum
                                        # (32 triggers PE 32-row-tile
                                        # transition crashes)
MUL_DVE = 0.0                           # fraction of multiply on DVE

_Q2 = 2.0 * math.exp(-math.pi ** 2 / 2.0)

_COMPILED = None          # (nc, plan, lay)


def _block_plan(x, y):
    """Assign blocks to (core, slot) by count-balanced dealing."""
    xp = (x.astype(np.float64) + 1.0) * (WIDTH / 2.0)
    yp = (y.astype(np.float64) + 1.0) * (HEIGHT / 2.0)
    xb = np.clip(np.floor(xp).astype(np.int64), 0, WIDTH - 1)
    yb = np.clip(np.floor(yp).astype(np.int64), 0, HEIGHT - 1)
    gb = (yb // BLK) * GRID + xb // BLK            # global block id
    counts = np.bincount(gb, minlength=NBLK)
    order = np.argsort(-counts, kind="stable")     # blocks by count desc
    core_of = np.empty(NBLK, np.int64)
    slot_of = np.empty(NBLK, np.int64)
    rank = np.arange(NBLK)
    core_of[order] = rank % N_CORES
    slot_of[order] = rank // N_CORES
    # slot capacity = max count within the slot's 8 blocks, CAPQ-quantized
    slot_max = counts[order].reshape(NSLOT, N_CORES).max(axis=1)
    caps = (np.ceil(slot_max / CAPQ).astype(np.int64) * CAPQ).clip(CAPQ, None)
    # inverse table: (core, slot) -> block id
    inv = np.empty((N_CORES, NSLOT), np.int64)
    inv[core_of[order], slot_of[order]] = order
    return dict(counts=counts, core_of=core_of, slot_of=slot_of,
                caps=caps, inv=inv)


def _layout_from_caps(caps):
    """Slot slab layout: slots packed per strip (8 slots/strip), strips
    padded to whole 128-slot columns.  Matmul segments: p0==64 -> k<=64.
    Each slot's segment list is rotated so a p0==0 segment (if any)
    comes first and carries start=True; slots with no p0==0 segment are
    marked need_zero (opened by a k=1 zero matmul)."""
    strip_cols = np.zeros(NSTRIP, np.int64)
    slot_off = np.zeros(NSLOT, np.int64)     # slot offset within strip
    jpos = np.zeros(NSLOT, np.int64)         # position of slot in strip
    col_base = np.zeros(NSTRIP, np.int64)
    segs = []
    for s in range(NSTRIP):
        off = 0
        for j in range(8):
            sl = s * 8 + j
            slot_off[sl] = off
            jpos[sl] = j
            off += int(caps[sl])
        strip_cols[s] = (off + 127) // 128
    col_base[1:] = np.cumsum(strip_cols)[:-1]
    F = int(strip_cols.sum())

    nzero = 0
    for s in range(NSTRIP):
        slist = []
        for sl in range(s * 8, s * 8 + 8):
            pos = int(slot_off[sl])
            rem = int(caps[sl])
            j = int(jpos[sl])
            parts = []
            while rem > 0:
                t = pos // 128
                p0 = pos % 128
                assert p0 in (0, 64), f"illegal partition base {p0}"
                k = min(128 - p0, rem)
                if p0 == 64:
                    k = min(k, 64)
                pos += k
                rem -= k
                parts.append((t, p0, k))
            # rotate a p0==0 part to the front (group must start at
            # partition base 0)
            i0 = next((i for i, e in enumerate(parts) if e[1] == 0), None)
            if i0 is None:
                nzero += 1
                need_zero = True
            else:
                parts = parts[i0:i0 + 1] + parts[:i0] + parts[i0 + 1:]
                need_zero = False
            n = len(parts)
            for i, (t, p0, k) in enumerate(parts):
                first = (i == 0) and not need_zero
                slist.append((t, p0, k, j, first, i == n - 1,
                              need_zero and i == 0))
        segs.append(slist)

    chunks = []
    s0 = 0
    while s0 < NSTRIP:
        s1 = s0
        cols = 0
        while s1 < NSTRIP and cols + strip_cols[s1] <= GCMAX:
            cols += strip_cols[s1]
            s1 += 1
        assert s1 > s0, "single strip exceeds GCMAX"
        chunks.append((s0, s1, int(col_base[s0]), int(cols)))
        s0 = s1
    return dict(slot_off=slot_off, strip_cols=strip_cols, col_base=col_base,
                jpos=jpos, F=F, segs=segs, chunks=chunks, nzero=nzero)


def _build_program(lay):
    import concourse.bacc as bacc
    import concourse.mybir as mybir
    from concourse.tile import TileContext

    dt = mybir.dt
    Act = mybir.ActivationFunctionType
    Alu = mybir.AluOpType

    F = lay["F"]
    nc = bacc.Bacc("TRN2", target_bir_lowering=False, debug=False)

    # dcq2: per chunk, [dcy cols | dcx cols] pair-duplicated -> [P, 2F, 2]
    dcq2 = nc.dram_tensor("dcq2", [P, 2 * F, 2], dt.int16,
                          kind="ExternalInput")
    # vnb2: F columns pair-duplicated -> [P, F, 2]
    vnb2 = nc.dram_tensor("vnb2", [P, F, 2], dt.bfloat16,
                          kind="ExternalInput")
    iot = nc.dram_tensor("iot", [P, 18, 2], dt.int16, kind="ExternalInput")
    out = nc.dram_tensor("out", [NSTRIP, PW, 8 * PW], dt.float32,
                         kind="ExternalOutput")

    SC = float(math.sqrt(2.0) / 256.0)

    with TileContext(nc) as tc:
        with (
            tc.tile_pool(name="io", bufs=1) as io,
            tc.tile_pool(name="prof", bufs=1) as prof,
            tc.tile_pool(name="stage", bufs=8) as stage,
            tc.tile_pool(name="psum", bufs=8, space="PSUM") as psum,
        ):
            t_dcq2 = io.tile([P, 2 * F, 2], dt.int16)
            t_vnb2 = io.tile([P, F, 2], dt.bfloat16)
            t_iot = io.tile([P, 18, 2], dt.int16)
            t_zro = io.tile([P, PW], dt.bfloat16)
            nc.sync.dma_start(out=t_dcq2[:], in_=dcq2[:])
            nc.sync.dma_start(out=t_vnb2[:], in_=vnb2[:])
            nc.sync.dma_start(out=t_iot[:], in_=iot[:])
            nc.gpsimd.memset(t_zro[:], 0.0)

            def emit_mm(ck):
                """Matmuls for a chunk's strips; returns strip tiles."""
                (s0, s1, c0, gc), pr, colp = ck
                strips = []
                for s in range(s0, s1):
                    strip = psum.tile([PW, 8 * PW], dt.float32,
                                      tag="strip", name="strip")
                    base = int(lay["col_base"][s]) - c0
                    for (t, p0, k, j, first, last, zopen) in lay["segs"][s]:
                        tl = base + t
                        if zopen:
                            nc.tensor.matmul(
                                out=strip[:, j * PW:(j + 1) * PW],
                                lhsT=t_zro[0:1, :],
                                rhs=t_zro[0:1, :],
                                start=True, stop=False)
                        nc.tensor.matmul(
                            out=strip[:, j * PW:(j + 1) * PW],
                            lhsT=pr[p0:p0 + k, tl],
                            rhs=colp[p0:p0 + k, tl],
                            start=first, stop=last)
                    strips.append((s, strip))
                return strips

            ncopy = [0]

            def emit_copy(strips):
                """Strip copies (alternating DVE/ACT) + output DMAs."""
                for s, strip in strips:
                    st = stage.tile([PW, 8 * PW], dt.float32,
                                    tag="st", name="st")
                    if ncopy[0] % 2 == 0:
                        nc.vector.tensor_copy(out=st[:], in_=strip[:])
                    else:
                        nc.scalar.copy(out=st[:], in_=strip[:])
                    ncopy[0] += 1
                    nc.sync.dma_start(out=out[s], in_=st[:])

            co2 = 0                     # running column offset into dcq2
            prev = None                 # software pipeline: mm lags 1 chunk
            pcopy = None                # copies lag 2 chunks
            for (s0, s1, c0, gc) in lay["chunks"]:
                d = prof.tile([P, 2 * gc, 18, 2], dt.int16, tag="d", bufs=2,
                              name=f"d{gc}")
                nc.vector.tensor_tensor(
                    out=d[:],
                    in0=t_iot[:, None, :, :].to_broadcast([P, 2 * gc, 18, 2]),
                    in1=t_dcq2[:, co2:co2 + 2 * gc, None, :]
                        .to_broadcast([P, 2 * gc, 18, 2]),
                    op=Alu.subtract)
                pr = prof.tile([P, 2 * gc, 18, 2], dt.bfloat16, tag="pr",
                               bufs=3, name=f"pr{gc}")
                nc.scalar.activation(out=pr[:], in_=d[:],
                                     func=Act.Derivative_Erf, scale=SC)
                colp = prof.tile([P, gc, 18, 2], dt.bfloat16, tag="colp",
                                 bufs=3, name=f"colp{gc}")
                # multiply split DVE/Pool for engine balance (by column)
                g1 = max(0, min(gc, int(round(gc * MUL_DVE))))
                if g1 > 0:
                    nc.vector.tensor_tensor(
                        out=colp[:, :g1], in0=pr[:, gc:gc + g1],
                        in1=t_vnb2[:, c0:c0 + g1, None, :]
                            .to_broadcast([P, g1, 18, 2]),
                        op=Alu.mult)
                if g1 < gc:
                    nc.gpsimd.tensor_tensor(
                        out=colp[:, g1:], in0=pr[:, gc + g1:2 * gc],
                        in1=t_vnb2[:, c0 + g1:c0 + gc, None, :]
                            .to_broadcast([P, gc - g1, 18, 2]),
                        op=Alu.mult)

                if pcopy is not None:
                    emit_copy(pcopy)
                    pcopy = None
                if prev is not None:
                    pcopy = emit_mm(prev)
                prev = ((s0, s1, c0, gc), pr, colp)
                co2 += 2 * gc
            if pcopy is not None:
                emit_copy(pcopy)
            emit_copy(emit_mm(prev))
    nc.compile()
    from concourse.bass_interp import get_hw_module
    nc.m = get_hw_module(nc.m)
    return nc


def _host_shard(x, y, values, plan, lay):
    from ml_dtypes import bfloat16

    xp = (x.astype(np.float64) + 1.0) * (WIDTH / 2.0)
    yp = (y.astype(np.float64) + 1.0) * (HEIGHT / 2.0)
    xb = np.clip(np.floor(xp).astype(np.int64), 0, WIDTH - 1)
    yb = np.clip(np.floor(yp).astype(np.int64), 0, HEIGHT - 1)
    bcx = xb // BLK
    bry = yb // BLK
    gb = bry * GRID + bcx
    core = plan["core_of"][gb]
    slot = plan["slot_of"][gb]
    dcxq_all = np.round((xp - (bcx * BLK - 2)) * 256).astype(np.int16)
    dcyq_all = np.round((yp - (bry * BLK - 2)) * 256).astype(np.int16)
    fxq = (dcxq_all.astype(np.int64) % 256) / 256.0
    fyq = (dcyq_all.astype(np.int64) % 256) / 256.0
    vnorm = (values.astype(np.float64)
             / (2.0 * (1.0 + _Q2 * np.cos(2 * np.pi * fxq))
                * (1.0 + _Q2 * np.cos(2 * np.pi * fyq))))

    F = lay["F"]
    # global slot slab base: strip col_base*128 + slot_off
    slab = lay["col_base"][slot // 8] * 128 + lay["slot_off"][slot]

    iota_a = np.broadcast_to((np.arange(PW, dtype=np.int16) * 256)[None, :],
                             (P, PW)).reshape(P, 18, 2).copy()

    in_maps = []
    for c in range(N_CORES):
        m = core == c
        ps = slot[m]
        order = np.argsort(ps, kind="stable")
        ps = ps[order]
        counts = np.bincount(ps, minlength=NSLOT)
        if (counts > plan["caps"]).any():
            raise RuntimeError("slot overflow vs caps")
        starts = np.zeros(NSLOT, np.int64)
        np.cumsum(counts[:-1], out=starts[1:])
        idx = np.arange(ps.size) - starts[ps]
        dst = slab[m][order] + idx

        ya = np.full(F * P, 18 * 256, np.int16)
        xa = np.full(F * P, 18 * 256, np.int16)
        va = np.zeros(F * P, np.float64)
        ya[dst] = dcyq_all[m][order]
        xa[dst] = dcxq_all[m][order]
        va[dst] = vnorm[m][order]

        yaT = ya.reshape(F, P).T            # [P, F]
        xaT = xa.reshape(F, P).T
        vaT = va.reshape(F, P).T

        # dcq2: per chunk [dcy cols | dcx cols], each value duplicated
        dcq2_a = np.empty((P, 2 * F, 2), np.int16)
        off = 0
        for (_, _, c0, gc) in lay["chunks"]:
            dcq2_a[:, off:off + gc, 0] = yaT[:, c0:c0 + gc]
            dcq2_a[:, off + gc:off + 2 * gc, 0] = xaT[:, c0:c0 + gc]
            off += 2 * gc
        dcq2_a[:, :, 1] = dcq2_a[:, :, 0]

        vnb2_a = np.empty((P, F, 2), bfloat16)
        vnb2_a[:, :, 0] = vaT.astype(bfloat16)
        vnb2_a[:, :, 1] = vnb2_a[:, :, 0]
        in_maps.append({"dcq2": dcq2_a, "vnb2": np.ascontiguousarray(vnb2_a),
                        "iot": iota_a})
    return in_maps


def _assemble(results, plan, lay):
    img = np.zeros((HEIGHT + 4, WIDTH + 4), np.float64)
    jpos = lay["jpos"]
    for c in range(N_CORES):
        strips = results[c]["out"]          # [NSTRIP, PW, 8*PW]
        for sl in range(NSLOT):
            gb = plan["inv"][c, sl]
            bry, bcx = divmod(int(gb), GRID)
            j = int(jpos[sl])
            patch = strips[sl // 8, :, j * PW:(j + 1) * PW]
            img[bry * BLK:bry * BLK + PW, bcx * BLK:bcx * BLK + PW] += patch
    return img[2:2 + HEIGHT, 2:2 + WIDTH].astype(np.float32)


def kernel(x, y, values):
    global _COMPILED
    if _COMPILED is None:
        plan = _block_plan(x, y)
        lay = _layout_from_caps(plan["caps"])
        nc = _build_program(lay)
        _COMPILED = (nc, plan, lay)
    nc, plan, lay = _COMPILED
    in_maps = _host_shard(x, y, values, plan, lay)
    from concourse.bass_utils import run_bass_kernel_spmd
    import os
    trace = bool(int(os.environ.get("SPLAT_TRACE", "0")))
    res = run_bass_kernel_spmd(nc, in_maps, list(range(N_CORES)), trace=trace)
    kernel.last_exec_time_ns = res.exec_time_ns
    kernel.last_results = res
    return _assemble(res.results, plan, lay)


kernel.last_exec_time_ns = None
